# revision 28
# baseline (speedup 1.0000x reference)
"""DifferentialAttention Trainium2 kernel (8 NeuronCores, SPMD).

Sharding: 2 batches x 4 token-blocks = 8 cores. Each core computes
projections + layernorm for its 512 tokens, AllGathers K/V within its
4-core (same-batch) group, then computes attention + output projection
for its own query tokens. Host stitches per-core [D, 512] outputs.

Layout strategy: everything "transposed" (features on partitions, tokens
on free dim) so the whole chain q-proj -> scores -> AV -> out-proj needs
zero on-device transposes:
  - LN mean subtraction is folded into host-side column-centering of the
    (ternary-quantized) weight matrices, and gamma is folded into the
    weight rows; only the variance needs device work (a matmul of the
    squared activations against a per-channel 1/(CH*gamma^2) stationary).
  - K's rstd is folded into kf in fp16 BEFORE the gather (costs ~4e-3
    logit noise, well inside tolerance), so the gathered K is fully
    normalized and the softmax exp needs NO per-key scale operand.
  - softmax runs without max-subtraction (scores are O(+-8); exp is safe
    in fp16) and the denominator comes free as a 65th "ones" column
    appended to V.
  - lambda (a host-computable scalar) rides in as a [1,1] input tensor so
    the compiled program is input-independent.

Schedule (the kernel is EXP-bound on the Scalar engine: 262144 exp
cycles/partition = 218us dense; everything is arranged around starting
that stream early and never letting it hiccup):
  - exps are FUSED 4 score-chunks per activation instruction
    ([128,4,512], scale-free thanks to the rstd fold) to amortize the
    ~290-cycle per-instruction ACT overhead: 128 instrs instead of 512.
  - phase order V-proj -> K-proj -> (gathers) -> Q-proj, with ALL weight
    strips (wv,wk,wq) loaded up front: kf stores now happen after the
    K-variance chain (~45us), long after the loads drain, so they don't
    queue.
  - the first attention pairs are gated by the CC stream init barrier
    (~64us) + per-gather link time (~27us/512KB), so the K/V gathers are
    split into 9 pieces issued in exactly consumption order:
    k0a1,k0a2,v0a,v0b,k1a1,k1a2,k0b,v1,k1b, with the pair loop running
    (b0,hp0-3),(b1,hp0-3),(b0,hp4-7),(b1,hp4-7). The first piece lands
    ~78us, right as Q's layernorm finishes.
  - variance matmuls lag the projection matmuls by one tile so the PE
    never waits on the Scalar engine's squares.
  - attention is branch-outer, and each (branch, head-pair) runs its
    score-matmuls + fused exps first, with the PREVIOUS pair's 16 AV
    matmuls interleaved - the in-order PE queue never waits on the
    ScalarE exp stream.

MM_DT picks the matmul operand dtype: float16 (default), bfloat16, or
float32r.
"""

import os
import sys
import types

for _p in ("/opt/trn_rl_repo",):
    if os.path.isdir(_p) and _p not in sys.path:
        sys.path.append(_p)

import numpy as np

import concourse.bass as bass
import concourse.tile as tile
from concourse.bass import _add_dep_helper
from concourse import bacc, mybir
from concourse.bass_utils import run_bass_kernel_spmd


def _install_ntff_shim():
    """bass_utils imports antenv.axon_hooks when tracing under axon; the
    container antenv stub lacks it. Back it with the ctypes hook."""
    if "antenv.axon_hooks" in sys.modules:
        return
    try:
        from trn_agent_boot.trn_boot import _ntff_profile_via_ctypes

        hook = _ntff_profile_via_ctypes("/opt/axon/libaxon_pjrt.so")
    except Exception:
        hook = None
    mod = types.ModuleType("antenv.axon_hooks")
    mod.get_axon_ntff_profile_hook = lambda: hook
    sys.modules["antenv.axon_hooks"] = mod


_install_ntff_shim()

# ----- problem dims (hardcoded per spec) -----
B, T, D = 2, 2048, 1024
H, DH = 16, 64
CH = 2 * H * DH  # 2048
EPS = 1e-5
NCORES = 8
GS = 4  # cores per batch group
GROUPS = [[0, 1, 2, 3], [4, 5, 6, 7]]

F32 = mybir.dt.float32
MM_DT = "f16"  # "f16" | "bf16" | "f32r"
_DT_MAP = {
    "f16": mybir.dt.float16,
    "bf16": mybir.dt.bfloat16,
    "f32r": mybir.dt.float32r,
}

_PROG_CACHE: dict = {}


def _bcast_part(ap, n):
    """AP view replicating a 1-partition AP across n partitions (step 0)."""
    return bass.AP(tensor=ap.tensor, offset=ap.offset, ap=[[0, n]] + list(ap.ap)[1:])


def build_program(t_total=T, has_beta=False, mm_dt=MM_DT, unit_gamma=True):
    """Build the per-core SPMD program. t_total is the per-batch sequence
    length (2048 for the real problem; smaller for simulation)."""
    TLOC = t_total // GS  # query tokens owned by this core
    KT = D // 128  # contraction tiles for projections
    CHT = CH // 128  # channel tiles of q/k
    NKC = t_total // 128  # key chunks
    TC = TLOC // 128  # token chunks (v projection)
    L = TLOC // 128  # 128-blocks per gather block
    HP = H // 2  # head pairs
    HPH = HP // 2  # head pairs per v-gather half
    DT = _DT_MAP[mm_dt]

    nc = bacc.Bacc("TRN2", target_bir_lowering=False, debug=False, num_devices=NCORES)

    # xT is host-prelaid as [128, KT*TLOC]: element [p, k*TLOC+t] = x^T[k*128+p, t]
    xT = nc.dram_tensor("xT", [128, KT * TLOC], DT, kind="ExternalInput").ap()
    wq_t = nc.dram_tensor("wq_t", [D, CH], DT, kind="ExternalInput").ap()
    wk_t = nc.dram_tensor("wk_t", [D, CH], DT, kind="ExternalInput").ap()
    wv_t = nc.dram_tensor("wv_t", [D, D], DT, kind="ExternalInput").ap()
    wo_t = nc.dram_tensor("wo_t", [D, D], DT, kind="ExternalInput").ap()
    wsq_q = wsq_k = None
    if not unit_gamma:
        wsq_q = nc.dram_tensor("wsq_q", [128, CH], DT, kind="ExternalInput").ap()
        wsq_k = nc.dram_tensor("wsq_k", [128, CH], DT, kind="ExternalInput").ap()
    lam_in = nc.dram_tensor("lam", [1, 1], F32, kind="ExternalInput").ap()
    ones_one_in = nc.dram_tensor("ones_one", [128, H], DT, kind="ExternalInput").ap()
    if has_beta:
        bq_in = nc.dram_tensor("bq", [128, CHT], F32, kind="ExternalInput").ap()
        bk_in = nc.dram_tensor("bk", [128, CHT], F32, kind="ExternalInput").ap()
    yT = nc.dram_tensor("yT", [D, TLOC], DT, kind="ExternalOutput").ap()

    # K gather pieces: (name, #channel-tiles). Each collective op costs ~6us
    # fixed on the serial CC stream, so quarters are the sweet spot - except
    # the FIRST piece, which gates the whole exp stream and is split in two
    # so the first kh strips land right after the stream init barrier.
    K_PIECES = [("0a1", 2), ("0a2", 2), ("0b", 4), ("1a", 4), ("1b", 4)]

    def k_route(t):
        """channel tile t -> (piece name, row-tile index within piece)"""
        b, hpp = t // 8, t % 8
        if b == 0 and hpp < 2:
            return ("0a1", hpp)
        if b == 0 and hpp < 4:
            return ("0a2", hpp - 2)
        if hpp < 4:
            return ("1a", hpp)
        return (f"{b}b", hpp - 4)

    def v_route(hp):
        """head-pair -> (piece name, index within piece)"""
        if hp < HPH:
            return ("v0", hp)
        return ("v1", hp - HPH)

    V_PIECES = [("v0", HPH), ("v1", HPH)]

    with tile.TileContext(nc) as tc:
        with (
            tc.tile_pool(name="const", bufs=1) as const,
            tc.tile_pool(name="dram", bufs=1, space="DRAM") as dram,
            tc.tile_pool(name="rdd_pool", bufs=4, space="DRAM") as rdd_pool,
            tc.tile_pool(name="qf_p", bufs=1) as qf_p,
            tc.tile_pool(name="attn_p", bufs=1) as attn_p,
            tc.tile_pool(name="wo_p", bufs=KT) as wo_p,
        ):
            # constants + tiny inputs
            ones_one = const.tile([128, H], DT)
            nc.sync.dma_start(ones_one[:], ones_one_in[:])
            lam_sb = const.tile([1, 1], F32)
            nc.sync.dma_start(lam_sb[:], lam_in[:])
            if unit_gamma:
                # gamma == 1: the variance stationary is the constant 1/CH
                # and every 128-column block is identical, so one memset
                # block replaces the 1MB wsq load
                wsq_blk = const.tile([128, 128], DT)
                nc.vector.memset(wsq_blk[:], 1.0 / CH)
                wsq_q_sb = wsq_k_sb = None
            else:
                wsq_q_sb = const.tile([128, CH], DT)
                nc.sync.dma_start(wsq_q_sb[:], wsq_q[:])
                wsq_k_sb = const.tile([128, CH], DT)
                nc.sync.dma_start(wsq_k_sb[:], wsq_k[:])

            def wsq_block(sb, t):
                return wsq_blk[:] if unit_gamma else sb[:, t * 128 : (t + 1) * 128]
            bq_sb = bk_sb = None
            if has_beta:
                bq_sb = const.tile([128, CHT], F32)
                nc.sync.dma_start(bq_sb[:], bq_in[:])
                bk_sb = const.tile([128, CHT], F32)
                nc.sync.dma_start(bk_sb[:], bk_in[:])

            # DRAM bounce/gather buffers
            kT_l = {}
            kT_g = {}
            for nm, tiles in K_PIECES:
                kT_l[nm] = dram.tile([tiles * 128, TLOC], DT, name=f"kT_l{nm}")
                kT_g[nm] = dram.tile([GS, tiles * 128, TLOC], DT, name=f"kT_g{nm}")
            # vA: head-pair-major so attention reads are contiguous
            vA_l = {}
            vA_g = {}
            for nm, nhp in V_PIECES:
                vA_l[nm] = dram.tile([nhp, 128, L * 130], DT, name=f"vA_l{nm}")
                vA_g[nm] = dram.tile([GS, nhp, 128, L * 130], DT, name=f"vA_g{nm}")

            # ---------------- Phase A: projections + LN + gathers -------
            with (
                tc.tile_pool(name="w_p", bufs=KT) as w_p,
                tc.tile_pool(name="wq_p", bufs=KT) as wq_p,
                tc.tile_pool(name="wv_p", bufs=KT) as wv_p,
                tc.tile_pool(name="kprime_p", bufs=1) as kprime_p,
                tc.tile_pool(name="qprime_p", bufs=1) as qprime_p,
                tc.tile_pool(name="sq_p", bufs=3) as sq_p,
                tc.tile_pool(name="stat_p", bufs=1) as stat_p,
                tc.tile_pool(name="ev_p", bufs=12) as ev_p,
                tc.tile_pool(name="vaug_p", bufs=1) as vaug_p,
                tc.tile_pool(name="xp", bufs=1) as xp,
                tc.tile_pool(name="pp", bufs=1, space="PSUM") as pp,
            ):
                xT_sb = xp.tile([128, KT, TLOC], DT)
                nc.sync.dma_start(
                    xT_sb[:], xT.rearrange("p (k t) -> p k t", k=KT)
                )

                def load_strips(pool, dram_t, tag):
                    strips = []
                    dw = dram_t.shape[1]
                    for j in range(KT):
                        ws = pool.tile([128, dw], DT, tag=tag, name=f"w_{tag}{j}")
                        nc.sync.dma_start(ws[:], dram_t[j * 128 : (j + 1) * 128, :])
                        strips.append(ws)
                    return strips

                # all projection weights load up front in consumption order
                # (K first): the kf stores that gate the first gather happen
                # after the K-variance chain (~65us), clear of this traffic
                kstrips = load_strips(w_p, wk_t, "wk")
                vstrips = load_strips(wv_p, wv_t, "wv")
                qstrips = load_strips(wq_p, wq_t, "wq")

                # ---- K projection (first: the gathers it feeds are floored
                # by the CC stream init barrier at ~78us anyway, and V/Q
                # only gate later consumers) ----
                kprime = kprime_p.tile([128, CHT, TLOC], DT)
                k_var = pp.tile([128, TLOC], F32, tag="kvar", bufs=1, name="k_var")
                sq_tiles = [None] * CHT

                def k_tile(t):
                    ps = pp.tile([128, TLOC], F32, tag="proj", bufs=2)
                    for j in range(KT):
                        nc.tensor.matmul(
                            ps[:],
                            kstrips[j][:, t * 128 : (t + 1) * 128],
                            xT_sb[:, j, :],
                            start=(j == 0),
                            stop=(j == KT - 1),
                        )
                    nc.vector.tensor_copy(kprime[:, t, :], ps[:])
                    sq = sq_p.tile([128, TLOC], DT, tag="sq")
                    nc.scalar.square(sq[:], ps[:])
                    sq_tiles[t] = sq

                def k_var_mm(t):
                    nc.tensor.matmul(
                        k_var[:],
                        wsq_block(wsq_k_sb, t),
                        sq_tiles[t][:],
                        start=(t == 0),
                        stop=(t == CHT - 1),
                    )

                # lag the variance matmul one tile behind the projection so
                # the PE stream never waits on the Scalar engine's square
                for t in range(CHT):
                    k_tile(t)
                    if t > 0:
                        k_var_mm(t - 1)
                k_var_mm(CHT - 1)

                # rstd_k as a [128, TLOC] partition-broadcast tile (the wsq
                # stationary's columns are identical per partition), folded
                # into kf in fp16 pre-gather: the gathered K ships fully
                # normalized and the exp needs no scale operand
                veps_k = stat_p.tile([128, TLOC], F32, tag="vepsk")
                nc.vector.tensor_scalar_add(veps_k[:], k_var[:], EPS)
                rec_k = stat_p.tile([128, TLOC], F32, tag="reck")
                nc.vector.reciprocal(rec_k[:], veps_k[:])
                rstd_k = stat_p.tile([128, TLOC], F32, tag="rstdk")
                nc.scalar.sqrt(rstd_k[:], rec_k[:])
                rk16 = stat_p.tile([128, TLOC], DT, tag="rk16")
                nc.vector.tensor_copy(rk16[:], rstd_k[:])
                for t in range(CHT):
                    kfs = ev_p.tile([128, TLOC], DT, tag="kf", name=f"kf{t}")
                    nc.vector.tensor_mul(kfs[:], kprime[:, t, :], rk16[:])
                    if has_beta:
                        nc.vector.tensor_scalar_add(
                            kfs[:], kfs[:], bk_sb[:, t : t + 1]
                        )
                    knm, kt_i = k_route(t)
                    # store via the (idle) gpsimd ring: the sync ring is
                    # backlogged with weight loads at this point and the
                    # first gather's trigger waits on these stores
                    nc.gpsimd.dma_start(
                        kT_l[knm][kt_i * 128 : (kt_i + 1) * 128, :], kfs[:]
                    )

                # gathers, chained in exactly consumption order
                def make_cc(ins, outs):
                    return nc.gpsimd.collective_compute(
                        "AllGather",
                        mybir.AluOpType.bypass,
                        replica_groups=GROUPS,
                        ins=[ins],
                        outs=[outs],
                    )

                def chained_cc(prev, ins, outs):
                    cc = make_cc(ins, outs)
                    if prev is not None:
                        _add_dep_helper(cc.ins, prev.ins, sync=True, reason="cc order")
                    return cc

                # first gathers issued now (their kf-store inputs are
                # written); the v0/v1 gathers are issued AFTER the
                # V-projection writes their payload - collective input deps
                # only see writes that precede them in program order
                cc_prev = chained_cc(None, kT_l["0a1"][:], kT_g["0a1"][:])
                cc_prev = chained_cc(cc_prev, kT_l["0a2"][:], kT_g["0a2"][:])

                # ---- V projection (its v0 payload is only consumed once
                # the 0a gather clears the serial CC stream) ----
                vaug = vaug_p.tile([128, L, H, 65], DT)
                for c in range(TC):
                    vps = pp.tile([128, D], F32, tag="vproj", bufs=2)
                    for j in range(KT):
                        for n in range(D // 512):
                            nc.tensor.matmul(
                                vps[:, n * 512 : (n + 1) * 512],
                                xT_sb[:, j, c * 128 : (c + 1) * 128],
                                vstrips[j][:, n * 512 : (n + 1) * 512],
                                start=(j == 0),
                                stop=(j == KT - 1),
                            )
                    nc.vector.tensor_copy(
                        vaug[:, c, :, 0:64], vps[:].rearrange("p (h d) -> p h d", h=H)
                    )
                    nc.sync.dma_start(vaug[:, c, :, 64:65], ones_one[:])
                # write head-pair-major: [hp][128, L*130] contiguous
                for hp in range(HP):
                    vnm, vi = v_route(hp)
                    nc.sync.dma_start(
                        vA_l[vnm][vi, :, :].rearrange("p (l c) -> p l c", l=L),
                        vaug[:, :, 2 * hp : 2 * hp + 2, :].rearrange(
                            "p l h d -> p l (h d)"
                        ),
                    )

                # rest of the gather chain, in consumption order
                cc_prev = chained_cc(cc_prev, vA_l["v0"][:], vA_g["v0"][:])
                for nm in ("1a", "0b"):
                    cc_prev = chained_cc(cc_prev, kT_l[nm][:], kT_g[nm][:])
                cc_prev = chained_cc(cc_prev, vA_l["v1"][:], vA_g["v1"][:])
                cc_prev = chained_cc(cc_prev, kT_l["1b"][:], kT_g["1b"][:])

                # ---- Q projection (stays resident; LN applied on device) ----
                qf_sb = qf_p.tile([128, CHT, TLOC], DT)
                qprime = qprime_p.tile([128, CHT, TLOC], DT)
                q_var = pp.tile([128, TLOC], F32, tag="qvar", bufs=1, name="q_var")

                def q_tile(t):
                    ps = pp.tile([128, TLOC], F32, tag="proj", bufs=2)
                    for j in range(KT):
                        nc.tensor.matmul(
                            ps[:],
                            qstrips[j][:, t * 128 : (t + 1) * 128],
                            xT_sb[:, j, :],
                            start=(j == 0),
                            stop=(j == KT - 1),
                        )
                    nc.vector.tensor_copy(qprime[:, t, :], ps[:])
                    sq = sq_p.tile([128, TLOC], DT, tag="sq")
                    nc.scalar.square(sq[:], ps[:])
                    sq_tiles[t] = sq

                def q_var_mm(t):
                    nc.tensor.matmul(
                        q_var[:],
                        wsq_block(wsq_q_sb, t),
                        sq_tiles[t][:],
                        start=(t == 0),
                        stop=(t == CHT - 1),
                    )

                for t in range(CHT):
                    q_tile(t)
                    if t > 0:
                        q_var_mm(t - 1)
                q_var_mm(CHT - 1)

                veps = stat_p.tile([128, TLOC], F32, tag="veps")
                nc.vector.tensor_scalar_add(veps[:], q_var[:], EPS)
                rec = stat_p.tile([128, TLOC], F32, tag="rec")
                nc.vector.reciprocal(rec[:], veps[:])
                rstd = stat_p.tile([128, TLOC], F32, tag="rstd")
                nc.scalar.sqrt(rstd[:], rec[:])
                # rstd_q * DH^-0.5 in fp16: unlocks the fast 16-bit DVE mode
                # for the 16 qf scale ops on the exp-critical path
                rq16 = stat_p.tile([128, TLOC], DT, tag="rq16")
                nc.vector.tensor_scalar_mul(rq16[:], rstd[:], DH**-0.5)
                for t in range(CHT):
                    nc.vector.tensor_mul(qf_sb[:, t, :], qprime[:, t, :], rq16[:])
                    if has_beta:
                        nc.vector.tensor_scalar_add(
                            qf_sb[:, t, :], qf_sb[:, t, :], bq_sb[:, t : t + 1]
                        )

            wostrips = []

            def load_wostrips():
                # prefetch output-projection weights mid-attention, clear of
                # the gather window's DMA traffic
                for j in range(KT):
                    ws = wo_p.tile([128, D], DT, tag="wo", name=f"w_o{j}")
                    nc.sync.dma_start(ws[:], wo_t[j * 128 : (j + 1) * 128, :])
                    wostrips.append(ws)

            # ---------------- Phase B: attention ------------------------
            # exp instructions are fused 3 score-units ([128,3,512], F=1536)
            # to amortize the ~290-cycle ACT per-instruction overhead. PSUM
            # budget (8 banks): sc 2x3 + av 2x1 = 8, made possible by
            # draining the AV accumulation to SBUF at half-sweeps.
            NU = 2 * NKC  # score units (key-chunk x head-parity) per pair
            NT = (NU + 2) // 3  # sc tiles per pair
            with (
                tc.tile_pool(name="kh_p", bufs=4) as kh_p,
                tc.tile_pool(name="vh_p", bufs=3) as vh_p,
                tc.tile_pool(name="pt_p", bufs=24) as pt_p,
                tc.tile_pool(name="o1_p", bufs=4) as o1_p,
                tc.tile_pool(name="den_p", bufs=4) as den_p,
                tc.tile_pool(name="rdb_p", bufs=4) as rdb_p,
                tc.tile_pool(name="acc_p", bufs=4) as acc_p,
                tc.tile_pool(name="scp", bufs=2, space="PSUM") as scp,
                tc.tile_pool(name="avp", bufs=2, space="PSUM") as avp,
            ):
                attn_sb = attn_p.tile([128, HP, TLOC], DT)
                o1_tiles = {}

                def combine(st):
                    # accs: (accN [128,T] numerators E@0-63/O@64-127,
                    #        accDE [1,T], accDO [1,T] denominators)
                    bb, hpp, (accN, accDE, accDO) = st
                    rdb = rdb_p.tile([128, TLOC], F32, tag="rdb")
                    for parity, accD in ((0, accDE), (1, accDO)):
                        rd = den_p.tile([1, TLOC], F32, tag="rd")
                        nc.vector.reciprocal(rd[:], accD[:])
                        if bb == 1:
                            nc.vector.tensor_scalar_mul(
                                rd[:], rd[:], lam_sb[0:1, 0:1]
                            )
                        rdd = rdd_pool.tile([1, TLOC], F32, tag="rdd")
                        nc.sync.dma_start(rdd[:], rd[:])
                        nc.sync.dma_start(
                            rdb[parity * 64 : parity * 64 + 64, :],
                            _bcast_part(rdd[:], 64),
                        )
                    if bb == 0:
                        o1 = o1_p.tile([128, TLOC], F32, tag="o1", name=f"o1_{hpp}")
                        o1_tiles[hpp] = o1
                        nc.vector.tensor_mul(o1[:], accN[:], rdb[:])
                    else:
                        o1 = o1_tiles.pop(hpp)
                        o2 = rdb_p.tile([128, TLOC], F32, tag="o2")
                        nc.vector.tensor_mul(o2[:], accN[:], rdb[:])
                        nc.vector.tensor_sub(attn_sb[:, hpp, :], o1[:], o2[:])

                drain_insts = []  # last pair's psum->SBUF drain ops

                def prev_av(prev, st, c):
                    """AV matmuls for key-chunk c of the previous pair; the
                    psum group drains into SBUF f32 accumulators at sweep
                    end so 2 PSUM banks suffice."""
                    pb, php, ppts, pvh = prev
                    uE, uO = 2 * c, 2 * c + 1
                    ptE = ppts[uE // 3][:, uE % 3, :]
                    ptO = ppts[uO // 3][:, uO % 3, :]
                    if c == 0:
                        st["cur"] = (
                            avp.tile([65, TLOC], F32, tag="av", name="avE"),
                            avp.tile([65, TLOC], F32, tag="av", name="avO"),
                        )
                    curE, curO = st["cur"]
                    mmE = nc.tensor.matmul(
                        curE[:], pvh[:, c, 0:65], ptE,
                        start=(c == 0), stop=(c == NKC - 1),
                    )
                    mmO = nc.tensor.matmul(
                        curO[:], pvh[:, c, 65:130], ptO,
                        start=(c == 0), stop=(c == NKC - 1),
                    )
                    if c == 0 and drain_insts:
                        # explicit WAR: the group-reset matmul must not race
                        # the previous pair's drain reads of these banks
                        for mm in (mmE, mmO):
                            for dr in drain_insts:
                                _add_dep_helper(
                                    mm.ins, dr.ins, sync=True,
                                    reason="av drain before psum reuse",
                                )
                    if c == NKC - 1:
                        # drain into the partition-assembled SBUF accs; the
                        # E->0-63 / O->64-127 shift rides the PSUM-sourced
                        # copies (exempt from the SBUF same-start rule)
                        accN, accDE, accDO = st["accs"]
                        drain_insts[:] = [
                            nc.vector.tensor_copy(accN[0:64, :], curE[0:64, :]),
                            nc.vector.tensor_copy(accN[64:128, :], curO[0:64, :]),
                            nc.vector.tensor_copy(accDE[:], curE[64:65, :]),
                            nc.vector.tensor_copy(accDO[:], curO[64:65, :]),
                        ]

                # pair order matches gather completion order:
                # k0a*, v0*, k1a*, k0b, v1, k1b
                pairs = (
                    [(0, hp) for hp in range(HPH)]
                    + [(1, hp) for hp in range(HPH)]
                    + [(0, hp) for hp in range(HPH, HP)]
                    + [(1, hp) for hp in range(HPH, HP)]
                )
                prev = None  # (b, hp, pts, vh)
                for pi, (b, hp) in enumerate(pairs):
                    if pi == 10:
                        load_wostrips()
                    # kh first: DMA queues are in-order and vh blocks on the
                    # (later) v gather; vh-first stalls the kh loads that
                    # gate this pair's scores
                    knm, khp = k_route(b * 8 + hp)
                    kh = kh_p.tile([128, t_total], DT, tag="kh")
                    for g in range(GS):
                        nc.sync.dma_start(
                            kh[:, g * TLOC : (g + 1) * TLOC],
                            kT_g[knm][g, khp * 128 : (khp + 1) * 128, :],
                        )
                    vnm, vi = v_route(hp)
                    vh = vh_p.tile([128, NKC, 130], DT, tag="vh")
                    for g in range(GS):
                        nc.sync.dma_start(
                            vh[:, g * L : (g + 1) * L, :],
                            vA_g[vnm][g, vi, :, :].rearrange("p (l c) -> p l c", l=L),
                        )
                    qE = qf_sb[0:64, b * (CHT // 2) + hp, :]
                    qO = qf_sb[64:128, b * (CHT // 2) + hp, :]
                    # scores + fused exp for (b, hp), interleaved with the
                    # AV matmuls of the previous iteration: the in-order PE
                    # queue always has ready work and never waits on ACT.
                    st = None
                    if prev is not None:
                        st = {
                            "accs": (
                                acc_p.tile([128, TLOC], F32, tag="acc", name="accN"),
                                acc_p.tile([1, TLOC], F32, tag="accd", name="accDE"),
                                acc_p.tile([1, TLOC], F32, tag="accd", name="accDO"),
                            )
                        }
                    pts = []
                    done = 0
                    for i in range(NT):
                        u0 = 3 * i
                        nu = min(3, NU - u0)
                        sc = scp.tile([128, 3, TLOC], F32, tag="sc")
                        for s in range(nu):
                            u = u0 + s
                            c, par = u >> 1, u & 1
                            nc.tensor.matmul(
                                sc[:, s, :],
                                kh[64 * par : 64 * par + 64, c * 128 : (c + 1) * 128],
                                qE if par == 0 else qO,
                                start=True,
                                stop=True,
                            )
                        pt = pt_p.tile([128, 3, TLOC], DT, tag="pt")
                        nc.scalar.activation(
                            pt[:, 0:nu, :], sc[:, 0:nu, :],
                            mybir.ActivationFunctionType.Exp,
                        )
                        pts.append(pt)
                        if prev is not None:
                            target = min(NKC, (NKC * (i + 1) + NT - 1) // NT)
                            while done < target:
                                prev_av(prev, st, done)
                                done += 1
                    if prev is not None:
                        combine((prev[0], prev[1], st["accs"]))
                    prev = (b, hp, pts, vh)
                # flush the last iteration's AV + combine
                lst = {
                    "accs": (
                        acc_p.tile([128, TLOC], F32, tag="acc", name="laccN"),
                        acc_p.tile([1, TLOC], F32, tag="accd", name="laccDE"),
                        acc_p.tile([1, TLOC], F32, tag="accd", name="laccDO"),
                    )
                }
                for c in range(NKC):
                    prev_av(prev, lst, c)
                combine((prev[0], prev[1], lst["accs"]))

            # ---------------- Phase C: output projection ----------------
            with (
                tc.tile_pool(name="ye_p", bufs=3) as ye_p,
                tc.tile_pool(name="yp", bufs=2, space="PSUM") as yp,
            ):
                for dt_ in range(D // 128):
                    yps = yp.tile([128, TLOC], F32, tag="y")
                    for j in range(KT):
                        nc.tensor.matmul(
                            yps[:],
                            wostrips[j][:, dt_ * 128 : (dt_ + 1) * 128],
                            attn_sb[:, j, :],
                            start=(j == 0),
                            stop=(j == KT - 1),
                        )
                    ye = ye_p.tile([128, TLOC], DT, tag="ye")
                    nc.vector.tensor_copy(ye[:], yps[:])
                    nc.sync.dma_start(yT[dt_ * 128 : (dt_ + 1) * 128, :], ye[:])

    nc.compile()
    return nc


# ---------------- host-side preparation ----------------


def _quantize(W):
    W = np.asarray(W, dtype=np.float32)
    scale = np.clip(np.abs(W).mean(axis=1, keepdims=True), 1e-5, None)
    wq = np.clip(np.round(W / scale), -1.0, 1.0)
    return (wq * scale).astype(np.float32)


def prepare_inputs(
    x, Wq, Wk, Wv, Wo, lambda_q, lambda_k, qn_gamma, qn_beta, kn_gamma, kn_beta,
    mm_dt=MM_DT,
):
    """Host prep: quantize + center weights, fold gamma, per-core slices."""
    np_dt = mybir.dt.np(_DT_MAP[mm_dt])
    x = np.asarray(x, dtype=np.float32)
    t_total = x.shape[1]
    ch = 2 * H * DH
    cht = ch // 128
    kt = D // 128
    tloc = t_total // GS

    Wq_e = _quantize(Wq)
    Wk_e = _quantize(Wk)
    Wv_e = _quantize(Wv)
    Wo_e = _quantize(Wo)
    # fold LN mean-subtraction into column-centered weights, gamma into rows
    gq = np.asarray(qn_gamma, np.float32)
    gk = np.asarray(kn_gamma, np.float32)
    Wq_c = (Wq_e - Wq_e.mean(axis=0, keepdims=True)) * gq[:, None]
    Wk_c = (Wk_e - Wk_e.mean(axis=0, keepdims=True)) * gk[:, None]

    wq_t = np.ascontiguousarray(Wq_c.T).astype(np_dt)
    wk_t = np.ascontiguousarray(Wk_c.T).astype(np_dt)
    wv_t = np.ascontiguousarray(Wv_e.T).astype(np_dt)
    wo_t = np.ascontiguousarray(Wo_e.T).astype(np_dt)

    def wsq_full(g):
        # stationary for the variance matmul: [128, cht*128] where column
        # block t is constant per-partition 1/(ch * gamma[t*128+p]^2)
        w = 1.0 / (ch * np.maximum(g, 1e-12) ** 2)
        return np.ascontiguousarray(
            np.repeat(w.reshape(cht, 128).T[:, :, None], 128, axis=2).reshape(
                128, cht * 128
            )
        ).astype(np_dt)

    lam = np.clip(
        np.exp(np.asarray(lambda_q).mean() - np.asarray(lambda_k).mean()), 0.1, 2.0
    ).astype(np.float32)

    has_beta = bool(np.any(np.asarray(qn_beta)) or np.any(np.asarray(kn_beta)))
    unit_gamma = bool(np.all(gq == 1.0) and np.all(gk == 1.0))
    common = {
        "wq_t": wq_t,
        "wk_t": wk_t,
        "wv_t": wv_t,
        "wo_t": wo_t,
        "lam": lam.reshape(1, 1),
        "ones_one": np.ones((128, H), np_dt),
    }
    if not unit_gamma:
        common["wsq_q"] = wsq_full(gq)
        common["wsq_k"] = wsq_full(gk)
    if has_beta:
        bq = np.asarray(qn_beta, np.float32) * (DH**-0.5)
        bk = np.asarray(kn_beta, np.float32)
        common["bq"] = np.ascontiguousarray(bq.reshape(cht, 128).T)
        common["bk"] = np.ascontiguousarray(bk.reshape(cht, 128).T)

    in_maps = []
    for c in range(NCORES):
        b = c // GS
        ts = c % GS
        xt = np.ascontiguousarray(x[b, ts * tloc : (ts + 1) * tloc, :].T).astype(np_dt)
        # pre-lay [D, tloc] -> [128, kt*tloc] so the device load is one
        # contiguous [128, 8KB] DMA instead of 1024 1KB descriptors
        xt = np.ascontiguousarray(
            xt.reshape(kt, 128, tloc).transpose(1, 0, 2).reshape(128, kt * tloc)
        )
        in_maps.append({**common, "xT": xt})
    return in_maps, has_beta, t_total, unit_gamma


def get_program(t_total=T, has_beta=False, mm_dt=MM_DT, unit_gamma=True):
    key = (t_total, has_beta, mm_dt, unit_gamma)
    if key not in _PROG_CACHE:
        _PROG_CACHE[key] = build_program(t_total, has_beta, mm_dt, unit_gamma)
    return _PROG_CACHE[key]


def run(inputs, trace=False, mm_dt=MM_DT):
    """Run on hardware; returns (full_output, BassKernelResults)."""
    in_maps, has_beta, t_total, unit_gamma = prepare_inputs(**inputs, mm_dt=mm_dt)
    nc = get_program(t_total, has_beta, mm_dt, unit_gamma)
    res = run_bass_kernel_spmd(nc, in_maps, list(range(NCORES)), trace=trace)
    tloc = t_total // GS
    out = np.empty((B, t_total, D), dtype=np.float32)
    for c in range(NCORES):
        b = c // GS
        ts = c % GS
        out[b, ts * tloc : (ts + 1) * tloc, :] = res.results[c]["yT"].T.astype(np.float32)
    return out, res


def kernel(**inputs) -> np.ndarray:
    out, _ = run(inputs, trace=False)
    return out
